# revision 1
# baseline (speedup 1.0000x reference)
"""Trainium2 Bass kernel for the 2-layer GATv2 + MLP-head model (nn_GAT_21028159881586).

Strategy (8 NeuronCores, SPMD single NEFF), ~4.5x faster than the previous
baseline:
  * Destination-block partitioning as v1: core c owns dst slice of 3750 nodes
    (padded 3840 = 30 windows x 128), so segment softmax is core-local.
  * Per layer: node transforms on the local slice, AllGather of xl, then per
    destination window of 128:
      - ONE dma_gather of xl rows in (edge, channel) layout (the only per-edge
        gather; 4 SWDGE queues round-robin so descriptor generation overlaps),
      - xr side via PE: the host ships dense 0/1 scatter matrices s_T (dst x
        edge) and s_t (edge x dst) per window; xr_edge = s_T.T @ xr_win in
        PSUM, m = gathered + psum (DVE), leaky-relu (ACT Prelu),
      - logits on DVE: lr * att_rep then tensor_reduce over each head's
        channel block; exp on ACT writes straight into the [gw | exp] rhs,
      - gw = gathered * exp (DVE broadcast), one PE matmul per 128-edge tile
        accumulates [aggregation | softmax denominator] into PSUM via s_t,
      - normalize, +bias, ELU, write the 128 output rows.
  * Padded edges have all-zero rows/cols in s_t/s_T so they contribute
    nothing regardless of gathered garbage - no edge bias needed.
  * MLP head: batch rows are assigned to the core owning their var node.

fp16 data, fp32 PSUM accumulation.
"""

import numpy as np

import concourse.bacc as bacc
import concourse.tile as tile
import concourse.mybir as mybir
from concourse.bass_utils import run_bass_kernel_spmd

fp8 = mybir.dt.float8e4

P = 128
NCORES = 8
N = 30000
NLOC_REAL = 3750
WIN = 30
NLOC = WIN * P            # 3840
NALL = NCORES * NLOC      # 30720
IN_DIM = 1281
KCH = 11
KPAD = KCH * P            # 1408
HID = 256
HEADS1 = 4
BLOC = 640
NEG = 0.2
CH0 = 23 * P              # AllGather chunk sizes (rows): big chunk
CH1 = 7 * P               # small tail chunk
CH = (CH0, CH1)

f32 = mybir.dt.float32
f16 = mybir.dt.float16
i16 = mybir.dt.int16
AF = mybir.ActivationFunctionType
OP = mybir.AluOpType
AX = mybir.AxisListType

_nc_cache = {}


def _wrap16(idx2d: np.ndarray) -> np.ndarray:
    """(W, E) int -> (W*128, E//16) int16, wrapped in 16 partitions, replicated
    across the 8 gpsimd cores."""
    w, e = idx2d.shape
    assert e % 16 == 0
    t = idx2d.reshape(w, e // 16, 16).transpose(0, 2, 1)
    return np.tile(t, (1, 8, 1)).reshape(w * P, e // 16).astype(np.int16)


def _preprocess(inputs):
    x = np.asarray(inputs["x"], np.float32)
    ei = np.asarray(inputs["edge_index"]).astype(np.int64)
    var_idx = np.asarray(inputs["var_node_idx"]).astype(np.int64)
    wt = np.asarray(inputs["wt_onehot"], np.float32)
    mut = np.asarray(inputs["mut_onehot"], np.float32)

    src = np.concatenate([ei[0], np.arange(N, dtype=np.int64)])
    dst = np.concatenate([ei[1], np.arange(N, dtype=np.int64)])
    # chunk-major padded global id: [chunk][rank][row] so each AllGather chunk
    # lands in a contiguous slice of its half-table
    s_r = src // NLOC_REAL
    s_l = src % NLOC_REAL
    src_pad = np.where(s_l < CH0,
                       s_r * CH0 + s_l,
                       NCORES * CH0 + s_r * CH1 + (s_l - CH0))

    order = np.argsort(dst, kind="stable")
    src_pad = src_pad[order]
    dst_s = dst[order]

    core_of = dst_s // NLOC_REAL
    dloc = dst_s - core_of * NLOC_REAL
    win_of = dloc // P

    chunk_of = (src_pad >= NCORES * CH0).astype(np.int64)
    flat = (core_of * WIN + win_of) * 2 + chunk_of
    counts = np.bincount(flat, minlength=NCORES * WIN * 2).reshape(-1, 2)
    T0 = int((counts[:, 0].max() + P - 1) // P)
    T1 = int((counts[:, 1].max() + P - 1) // P)
    ew = (T0 + T1) * P
    T = T0 + T1

    per_core = []
    for c in range(NCORES):
        sel = core_of == c
        sp_c, dl_c, w_c = src_pad[sel], dloc[sel], win_of[sel]
        ch_c = chunk_of[sel]
        srcw = np.zeros((WIN, ew), np.int64)
        drlw = np.full((WIN, ew), -1, np.int64)     # -1 => padding edge
        for w in range(WIN):
            for ck in range(2):
                m = (w_c == w) & (ch_c == ck)
                k = int(m.sum())
                o = np.argsort(sp_c[m], kind="stable")  # HBM locality
                base = ck * T0 * P
                srcw[w, base:base + k] = sp_c[m][o] - ck * NCORES * CH0
                drlw[w, base:base + k] = dl_c[m][o] - w * P
        si = _wrap16(srcw)                          # (WIN*P, ew//16) i16
        # s_t[w*128+p, t*128+d] = 1 if drl[w, t*128+p] == d  (edge-part, dst)
        dr_pt = drlw.reshape(WIN, T, P).transpose(0, 2, 1)     # [w, p, t]
        s_t = (dr_pt[:, :, :, None] == np.arange(P)[None, None, None, :])
        s_t = s_t.reshape(WIN * P, T * P)
        # s_T[w*128+d, t*128+e] = 1 if drl[w, t*128+e] == d   (dst-part, edge)
        dr_te = drlw.reshape(WIN, T, P)                        # [w, t, e]
        s_T = (np.arange(P)[None, :, None, None] == dr_te[:, None, :, :])
        s_T = s_T.reshape(WIN, P, T * P)
        per_core.append(dict(si=si,
                     s_t=s_t.astype(np.float32).astype(mybir.dt.np(fp8)),
                     s_T=s_T.reshape(WIN * P, T * P).astype(np.float32).astype(mybir.dt.np(fp8))))

    # ---- shared weights / constants
    def pad_kT(w, m):
        wp = np.zeros((KPAD, m), np.float32)
        wp[:IN_DIM] = w
        return wp.reshape(KCH, P, m).transpose(1, 0, 2).reshape(P, KCH * m).astype(np.float16)

    def two_chunk(w):
        m = w.shape[1]
        return w.reshape(2, P, m).transpose(1, 0, 2).reshape(P, 2 * m).astype(np.float16)

    # att replicated to full window width (contiguous DVE multiply)
    att1 = np.asarray(inputs["att1"], np.float32)           # (4, 64)
    attrep1 = np.broadcast_to(np.tile(att1.reshape(1, HID), (1, T)),
                              (P, T * HID)).copy().astype(np.float16)
    attrep2 = np.broadcast_to(np.tile(np.asarray(inputs["att2"], np.float32
                                                 ).reshape(1, HID), (1, T)),
                              (P, T * HID)).copy().astype(np.float16)

    def rep_bias(b):
        return np.broadcast_to(np.asarray(b, np.float32)[None, :], (P, HID)).copy()

    hW1 = np.asarray(inputs["hW1"], np.float32)
    wlr1 = np.concatenate([np.asarray(inputs["Wl1"], np.float32),
                           np.asarray(inputs["Wr1"], np.float32)], axis=1)
    wlr2 = np.concatenate([np.asarray(inputs["Wl2"], np.float32),
                           np.asarray(inputs["Wr2"], np.float32)], axis=1)
    shared = dict(
        wlr1=pad_kT(wlr1, 2 * HID),
        wlr2=two_chunk(wlr2),
        attrep1=attrep1,
        attrep2=attrep2,
        blr1=np.concatenate([rep_bias(inputs["bl1"]), rep_bias(inputs["br1"])], 1),
        bias1=rep_bias(inputs["bias1"]),
        blr2=np.concatenate([rep_bias(inputs["bl2"]), rep_bias(inputs["br2"])], 1),
        bias2=rep_bias(inputs["bias2"]),
        hw1a=hW1[0:128].astype(np.float16),
        hw1b=hW1[128:256].astype(np.float16),
        hw1c=np.vstack([hW1[256:296], np.zeros((8, 128), np.float32)]).astype(np.float16),
        hw2=np.asarray(inputs["hW2"], np.float32).astype(np.float16),
        hw3=np.asarray(inputs["hW3"], np.float32).astype(np.float16),
        hb1=np.asarray(inputs["hb1"], np.float32).reshape(P, 1),
        hb2=np.asarray(inputs["hb2"], np.float32).reshape(64, 1),
        hb3=np.asarray(inputs["hb3"], np.float32).reshape(1, 1),
        ident=np.eye(P, dtype=np.float16),
    )

    # ---- per-core x slices, transposed + padded, chunked layout
    for c in range(NCORES):
        xp = np.zeros((KPAD, NLOC), np.float32)
        xp[:IN_DIM, :NLOC_REAL] = x[c * NLOC_REAL:(c + 1) * NLOC_REAL].T
        per_core[c]["xt"] = xp.reshape(KCH, P, NLOC).transpose(1, 0, 2).reshape(
            P, KCH * NLOC).astype(np.float16)

    # ---- MLP batch assignment
    vcore = var_idx // NLOC_REAL
    vloc = var_idx - vcore * NLOC_REAL
    batch_rows = []
    for c in range(NCORES):
        rows = np.nonzero(vcore == c)[0]
        assert len(rows) <= BLOC, f"core {c} has {len(rows)} batch rows > {BLOC}"
        batch_rows.append(rows)
        vi = np.zeros((1, BLOC), np.int64)
        vi[0, :len(rows)] = vloc[rows]
        per_core[c]["varloc"] = _wrap16(vi)
        wm = np.zeros((40, BLOC), np.float32)
        wm[:20, :len(rows)] = wt[rows].T
        wm[20:, :len(rows)] = mut[rows].T
        per_core[c]["wtmut"] = wm.astype(np.float16)

    return per_core, shared, batch_rows, (ew, T0 * P)


def _build(ew_t0, no_collectives=False):
    ew, ewc0 = ew_t0
    T = ew // P
    TC0 = ewc0 // P
    nc = bacc.Bacc("TRN2", target_bir_lowering=False, debug=False,
                   num_devices=1 if no_collectives else NCORES,
                   num_swdge_queues=4)

    io = {}
    io["xt"] = nc.dram_tensor("xt", [P, KCH * NLOC], f16, kind="ExternalInput")
    for nm, sh, dt in (
        ("wlr1", [P, KCH * 2 * HID], f16), ("wlr2", [P, 4 * HID], f16),
        ("attrep1", [P, T * HID], f16), ("attrep2", [P, T * HID], f16),
        ("blr1", [P, 2 * HID], f32), ("bias1", [P, HID], f32),
        ("blr2", [P, 2 * HID], f32), ("bias2", [P, HID], f32),
        ("hw1a", [P, P], f16), ("hw1b", [P, P], f16), ("hw1c", [48, P], f16),
        ("hw2", [P, 64], f16), ("hw3", [64, 1], f16),
        ("hb1", [P, 1], f32), ("hb2", [64, 1], f32), ("hb3", [1, 1], f32),
        ("si", [WIN * P, ew // 16], i16),
        ("s_t", [WIN * P, T * P], fp8), ("s_T", [WIN * P, T * P], fp8),
        ("varloc", [P, BLOC // 16], i16), ("wtmut", [40, BLOC], f16),
        ("ident", [P, P], f16),
    ):
        io[nm] = nc.dram_tensor(nm, sh, dt, kind="ExternalInput")
    out = nc.dram_tensor("out", [1, BLOC], f32, kind="ExternalOutput")

    with tile.TileContext(nc) as tc:
        with (
            tc.tile_pool(name="const", bufs=1) as cp,
            tc.tile_pool(name="dram", bufs=1, space="DRAM") as dr,
        ):
            c_ = {}
            for nm in ("wlr2", "attrep1", "attrep2", "bias1", "blr2", "bias2",
                       "hw1a", "hw1b", "hw1c", "hw2",
                       "hw3", "hb1", "hb2", "hb3", "varloc", "wtmut", "ident"):
                h = io[nm]
                c_[nm] = cp.tile(list(h.shape), h.dtype, tag=nm, name=f"c_{nm}")
                nc.sync.dma_start(c_[nm][:], h[:])

            xl1_loc = [dr.tile([CH[i], HID], f16, name=f"xl1_loc{i}") for i in range(2)]
            xr1_loc = dr.tile([NLOC, HID], f16)
            xl1_all = [dr.tile([CH[i] * NCORES, HID], f16, addr_space="Shared",
                               name=f"xl1_all{i}") for i in range(2)]
            h1_loc = dr.tile([NLOC, HID], f16)
            xl2_loc = [dr.tile([CH[i], HID], f16, name=f"xl2_loc{i}") for i in range(2)]
            xr2_loc = dr.tile([NLOC, HID], f16)
            xl2_all = [dr.tile([CH[i] * NCORES, HID], f16, addr_space="Shared",
                               name=f"xl2_all{i}") for i in range(2)]
            h2_loc = dr.tile([NLOC, HID], f16)

            # ---------- phase A layer 1 ----------
            with (
                tc.tile_pool(name="pa_sb", bufs=2) as sb,
                tc.tile_pool(name="pa_xt", bufs=1) as xp,
                tc.tile_pool(name="pa_ps", bufs=4, space="PSUM") as ps,
            ):
                xt = xp.tile([P, KCH, NLOC], f16)
                xtv = io["xt"][:].rearrange("p (k n) -> p k n", k=KCH)
                for xq in range(4):
                    n0, n1 = xq * (NLOC // 4), (xq + 1) * (NLOC // 4)
                    nc.sync.dma_start(xt[:, :, n0:n1], xtv[:, :, n0:n1])
                wlr1 = xp.tile([P, KCH, 2 * HID], f16)
                nc.sync.dma_start(wlr1[:], io["wlr1"][:].rearrange("p (k n) -> p k n", k=KCH))
                blr1 = xp.tile([P, 2 * HID], f32)
                nc.sync.dma_start(blr1[:], io["blr1"][:])
                for nt in range(WIN):
                    pa = ps.tile([P, 2 * HID], f32, tag="pa")
                    for k in range(KCH):
                        nc.tensor.matmul(pa[:], lhsT=xt[:, k, nt * P:(nt + 1) * P],
                                         rhs=wlr1[:, k, :],
                                         start=(k == 0), stop=(k == KCH - 1))
                    o = sb.tile([P, 2 * HID], f16, tag="pao")
                    nc.vector.tensor_tensor(out=o[:], in0=pa[:], in1=blr1[:],
                                            op=OP.add)
                    ck = 0 if nt * P < CH0 else 1
                    rr = nt * P - ck * CH0
                    nc.scalar.dma_start(xl1_loc[ck][rr:rr + P, :], o[:, 0:HID])
                    nc.scalar.dma_start(xr1_loc[nt * P:(nt + 1) * P, :], o[:, HID:2 * HID])
                    if nt == CH0 // P - 1:
                        nc.gpsimd.collective_compute(
                            "AllGather", OP.bypass,
                            replica_groups=[list(range(NCORES))],
                            ins=[xl1_loc[0][:].opt()], outs=[xl1_all[0][:].opt()])

            nc.gpsimd.collective_compute(
                "AllGather", OP.bypass, replica_groups=[list(range(NCORES))],
                ins=[xl1_loc[1][:].opt()], outs=[xl1_all[1][:].opt()])

            # layer-1 message passing with the layer-2 node transform
            # pipelined per window (phase B)
            with (
                tc.tile_pool(name="pb_sb", bufs=2) as pbsb,
                tc.tile_pool(name="pb_ht", bufs=2) as pbhp,
                tc.tile_pool(name="pb_ps", bufs=2, space="PSUM") as pbps,
            ):
                blr2 = c_["blr2"]

                def post_l1(nt):
                    rows = slice(nt * P, (nt + 1) * P)
                    ht = pbhp.tile([P, 2, P], f16, tag="ht")
                    for k in range(2):
                        nc.sync.dma_start_transpose(
                            ht[:, k, :], h1_loc[rows, k * P:(k + 1) * P])
                    pa = pbps.tile([P, 2 * HID], f32, tag="pb")
                    for k in range(2):
                        nc.tensor.matmul(
                            pa[:], lhsT=ht[:, k, :],
                            rhs=c_["wlr2"][:, k * 2 * HID:(k + 1) * 2 * HID],
                            start=(k == 0), stop=(k == 1))
                    o = pbsb.tile([P, 2 * HID], f16, tag="pbo")
                    nc.vector.tensor_tensor(out=o[:], in0=pa[:], in1=blr2[:],
                                            op=OP.add)
                    ck = 0 if nt * P < CH0 else 1
                    rr = nt * P - ck * CH0
                    nc.scalar.dma_start(xl2_loc[ck][rr:rr + P, :], o[:, 0:HID])
                    nc.scalar.dma_start(xr2_loc[rows, :], o[:, HID:2 * HID])
                    if nt == CH0 // P - 1:
                        nc.gpsimd.collective_compute(
                            "AllGather", OP.bypass,
                            replica_groups=[list(range(NCORES))],
                            ins=[xl2_loc[0][:].opt()], outs=[xl2_all[0][:].opt()])

                _emit_layer(nc, tc, ew=ew, TC0=TC0, heads=HEADS1, xl_all=xl1_all,
                            xr_loc=xr1_loc, h_out=h1_loc, attrep=c_["attrep1"],
                            bias_mat=c_["bias1"], io=io, ident=c_["ident"],
                            tag="l1", post_window=post_l1)

            nc.gpsimd.collective_compute(
                "AllGather", OP.bypass, replica_groups=[list(range(NCORES))],
                ins=[xl2_loc[1][:].opt()], outs=[xl2_all[1][:].opt()])

            _emit_layer(nc, tc, ew=ew, TC0=TC0, heads=1, xl_all=xl2_all,
                        xr_loc=xr2_loc, h_out=h2_loc, attrep=c_["attrep2"],
                        bias_mat=c_["bias2"], io=io, ident=c_["ident"], tag="l2")

            # ---------- MLP head ----------
            with (
                tc.tile_pool(name="mlp_sb", bufs=2) as sb,
                tc.tile_pool(name="mlp_ps", bufs=2, space="PSUM") as ps,
            ):
                sel = sb.tile([P, 2, BLOC], f16)
                nc.gpsimd.dma_gather(sel[:], h2_loc[:], c_["varloc"][:],
                                     num_idxs=BLOC, num_idxs_reg=BLOC,
                                     elem_size=HID, transpose=True)
                for c0, cn in ((0, 512), (512, BLOC - 512)):
                    z1p = ps.tile([P, 512], f32, tag="z1p")
                    nc.tensor.matmul(z1p[:, :cn], lhsT=c_["hw1a"][:],
                                     rhs=sel[:, 0, c0:c0 + cn], start=True, stop=False)
                    nc.tensor.matmul(z1p[:, :cn], lhsT=c_["hw1b"][:],
                                     rhs=sel[:, 1, c0:c0 + cn], start=False, stop=False)
                    nc.tensor.matmul(z1p[:, :cn], lhsT=c_["hw1c"][0:40, :],
                                     rhs=c_["wtmut"][:, c0:c0 + cn], start=False, stop=True)
                    z1 = sb.tile([P, 512], f16, tag="z1")
                    nc.scalar.activation(z1[:, :cn], z1p[:, :cn], AF.Relu,
                                         bias=c_["hb1"][:])
                    z2p = ps.tile([64, 512], f32, tag="z2p")
                    nc.tensor.matmul(z2p[:, :cn], lhsT=c_["hw2"][:],
                                     rhs=z1[:, :cn], start=True, stop=True)
                    z2 = sb.tile([64, 512], f16, tag="z2")
                    nc.scalar.activation(z2[:, :cn], z2p[:, :cn], AF.Relu,
                                         bias=c_["hb2"][:])
                    z3p = ps.tile([1, 512], f32, tag="z3p")
                    nc.tensor.matmul(z3p[:, :cn], lhsT=c_["hw3"][:],
                                     rhs=z2[:, :cn], start=True, stop=True)
                    z3 = sb.tile([1, 512], f32, tag="z3")
                    nc.scalar.activation(z3[:, :cn], z3p[:, :cn], AF.Identity,
                                         bias=c_["hb3"][:])
                    nc.sync.dma_start(out[0:1, c0:c0 + cn], z3[:, :cn])

    nc.compile()
    return nc


def _eighths_split(T, T0):
    out = []
    for hb, he in [(0, T0), (T0, T)]:
        n = he - hb
        parts = 5 if hb == 0 else 3
        cuts = [hb + (n * i) // parts for i in range(parts + 1)]
        out += [(a, b) for a, b in zip(cuts, cuts[1:]) if b > a]
    return out


def _eighths(T):
    out = []
    for hb, he in [(0, T // 2), (T // 2, T)]:
        n = he - hb
        cuts = [hb + (n * i) // 4 for i in range(5)]
        out += [(a, b) for a, b in zip(cuts, cuts[1:]) if b > a]
    return out


def _quarters(T):
    T2 = T // 2
    qa = T2 // 2
    qb = T2 - qa
    return [(0, qa), (qa, T2), (T2, T2 + qb), (T2 + qb, T)]


def _emit_layer(nc, tc, *, ew, TC0, heads, xl_all, xr_loc, h_out, attrep,
                bias_mat, io, ident, tag, post_window=None):
    T = ew // P
    CW = HID // heads
    NB = (T + 3) // 4
    r_q = {}
    for tb, te in _eighths_split(T, TC0):
        if te - tb not in r_q:
            r_q[te - tb] = nc.gpsimd.to_reg((te - tb) * P)
    with (
        tc.tile_pool(name=f"{tag}_g", bufs=8) as gp,
        tc.tile_pool(name=f"{tag}_s", bufs=2) as sp,
        tc.tile_pool(name=f"{tag}_si", bufs=8) as sip,
        tc.tile_pool(name=f"{tag}_w", bufs=3) as wp,
        tc.tile_pool(name=f"{tag}_e", bufs=2) as ep,
        tc.tile_pool(name=f"{tag}_pm", bufs=2, space="PSUM") as pmp,
        tc.tile_pool(name=f"{tag}_pa", bufs=2, space="PSUM") as pap,
    ):
        for w in range(WIN):
            rows = slice(w * P, (w + 1) * P)
            si = sip.tile([P, ew // 16], i16, tag="si")
            nc.sync.dma_start(si[:], io["si"][rows, :])
            sT = sp.tile([P, T, P], fp8, tag="sT")
            nc.sync.dma_start(sT[:], io["s_T"][rows, :].rearrange("p (t e) -> p t e", t=T))
            st = sp.tile([P, T, P], fp8, tag="st")
            nc.sync.dma_start(st[:], io["s_t"][rows, :].rearrange("p (t e) -> p t e", t=T))
            xrw = sp.tile([P, HID], f16, tag="xrw")
            nc.sync.dma_start(xrw[:], xr_loc[rows, :])

            gec = gp.tile([P, T, HID], f16, tag="gec")
            for j, (tb, te) in enumerate(_eighths_split(T, TC0)):
                ck = 0 if te <= TC0 else 1
                nc.gpsimd.dma_gather(gec[:, tb:te, :], xl_all[ck][:],
                                     si[:, tb * 8:te * 8],
                                     num_idxs=(te - tb) * P,
                                     num_idxs_reg=r_q[te - tb],
                                     elem_size=HID, transpose=False,
                                     single_packet=False,
                                     queue_num=(w + j) % 4)

            # m = xl[src] + xr[dst] in PSUM (xr via s_T matmul, xl via identity
            # matmul); lr = leaky_relu(m)   (edge, channel)
            lr = wp.tile([P, T, HID], f16, tag="lr")
            for b in range(NB):
                nb = min(4, T - 4 * b)
                pm = pmp.tile([P, 4, HID], f32, tag="pm")
                for tt in range(nb):
                    t = 4 * b + tt
                    nc.tensor.matmul(pm[:, tt, :], lhsT=sT[:, t, :], rhs=xrw[:],
                                     start=True, stop=False)
                    nc.tensor.matmul(pm[:, tt, :], lhsT=ident[:],
                                     rhs=gec[:, t, :], start=False, stop=True)
                nc.scalar.activation(lr[:, 4 * b:4 * b + nb, :], pm[:, 0:nb, :],
                                     AF.Prelu, alpha=NEG)

            # logits per head: pool-avg of lr * att over each head's channel
            # block (attrep is pre-scaled by CW on the host to undo the avg)
            lra = lr[:].rearrange("p t (h c) -> p t h c", h=heads)
            nc.vector.tensor_tensor(
                out=lr[:].rearrange("p t c -> p (t c)"),
                in0=lr[:].rearrange("p t c -> p (t c)"),
                in1=attrep[:], op=OP.mult)
            wdt = CW
            while wdt > 1:
                half = wdt // 2
                nc.vector.tensor_tensor(
                    out=lra[:, :, :, 0:half], in0=lra[:, :, :, 0:half],
                    in1=lra[:, :, :, half:wdt], op=OP.add)
                wdt = half

            gwx = wp.tile([P, T, HID + heads], f16, tag="gwx")
            nc.scalar.activation(
                gwx[:, :, HID:HID + heads].rearrange("p t (h o) -> p t h o", o=1),
                lra[:, :, :, 0:1], AF.Exp)
            T2g = T // 2
            for hw_ in range(2):
                ts_ = slice(hw_ * T2g, (hw_ + 1) * T2g if hw_ else T2g)
                tslice = slice(hw_ * T2g, T if hw_ else T2g)
                nt_ = tslice.stop - tslice.start
                nc.vector.tensor_tensor(
                    out=gwx[:, tslice, 0:HID].rearrange("p t (h c) -> p t h c", h=heads),
                    in0=gec[:, tslice, :].rearrange("p t (h c) -> p t h c", h=heads),
                    in1=gwx[:, tslice, HID:HID + heads].rearrange(
                        "p t (h o) -> p t h o", o=1).to_broadcast([P, nt_, heads, CW]),
                    op=OP.mult)

            agg = pap.tile([P, HID + heads], f32, tag="agg")
            for t in range(T):
                nc.tensor.matmul(agg[:], lhsT=st[:, t, :], rhs=gwx[:, t, :],
                                 start=(t == 0), stop=(t == T - 1))

            # normalize + bias + ELU
            den = ep.tile([P, heads], f32, tag="den")
            nc.vector.tensor_scalar_add(den[:], agg[:, HID:HID + heads], 1e-16)
            rden = ep.tile([P, heads], f32, tag="rden")
            nc.vector.reciprocal(rden[:], den[:])
            hb = ep.tile([P, HID], f32, tag="hb")
            nc.vector.tensor_tensor(
                out=hb[:].rearrange("p (h c) -> p h c", h=heads),
                in0=agg[:, 0:HID].rearrange("p (h c) -> p h c", h=heads),
                in1=rden[:].to_broadcast([P, heads, CW]), op=OP.mult)
            nc.vector.tensor_tensor(out=hb[:], in0=hb[:], in1=bias_mat[:], op=OP.add)
            # ELU: hb + r + exp(-r) - 1  with r = relu(-hb)
            r = ep.tile([P, HID], f32, tag="r")
            nc.scalar.activation(r[:], hb[:], AF.Relu, scale=-1.0)
            ex = ep.tile([P, HID], f32, tag="ex")
            nc.scalar.activation(ex[:], r[:], AF.Exp, scale=-1.0)
            t1 = ep.tile([P, HID], f32, tag="t1")
            nc.vector.tensor_tensor(out=t1[:], in0=hb[:], in1=r[:], op=OP.add)
            h_t = ep.tile([P, HID], f16, tag="h_t")
            nc.vector.scalar_tensor_tensor(out=h_t[:], in0=ex[:], scalar=-1.0,
                                           in1=t1[:], op0=OP.add, op1=OP.add)
            nc.scalar.dma_start(h_out[rows, :], h_t[:])
            if post_window is not None:
                post_window(w)


def kernel(**inputs):
    per_core, shared, batch_rows, ew = _preprocess(inputs)

    if ew not in _nc_cache:
        _nc_cache[ew] = _build(ew)
    nc = _nc_cache[ew]

    in_maps = []
    for c in range(NCORES):
        m = dict(shared)
        m.update(per_core[c])
        in_maps.append({k: np.ascontiguousarray(v) for k, v in m.items()})

    res = run_bass_kernel_spmd(nc, in_maps, core_ids=list(range(NCORES)))

    B = len(np.asarray(inputs["var_node_idx"]))
    out = np.zeros((B,), np.float32)
    for c in range(NCORES):
        rows = batch_rows[c]
        out[rows] = res.results[c]["out"][0, :len(rows)]
    return out

